# revision 35
# baseline (speedup 1.0000x reference)
"""Trainium2 Bass kernel for nn_ActorAction (moe_routing).

Computation (see reference):
  option_embed = embed_table[option]              [B, 64]
  all_state    = concat([state, option_embed])    [B, 576]
  cls_X = MLP_relu(all_state; Wx1,bx1,Wx2,bx2)    [B, 256]
  cls_Y = MLP_relu(all_state; Wy1,by1,Wy2,by2)    [B, 256]
  out_X = cls_X @ noise_lib_X                     [B, 256]
  out_Y[b] = cls_Y[b] @ noise_lib_Y[option[b]]    [B, 256]

Strategy: CLASS-sharded routing. Host ranks the 64 classes by count and
forms 8 rank-groups; group j's 8 classes go one-per-core into slot j,
so every core holds 8 whole classes and only needs THEIR noise_lib_Y
rows (1/8 of NY -> 1.05MB vs 8.4MB replicated). The Bass program is
shared SPMD: slot capacities c_j = max count in rank group j are
compile-time constants; per-core inputs (xt columns, ny slot payload)
differ. MLPs run feature-major (weights stationary, samples streamed)
chunk-major (both layers of a column-chunk finish before the next, so
routing / output DMAs start early); routing is ONE psum-wide matmul
pair per slot (c_j <= 128 rows), 16 matmuls total. Post-ops alternate
ScalarE/VectorE; outputs are written bf16 to halve output DMA; the
all-zero-bias case (true for setup_inputs) skips the bias input
entirely.

Schedule (from trace analysis):
- The gating input DMAs (w1y piece0 + xt) and the PE warmup matmuls are
  emitted BEFORE the TileContext, into the 'main' block, so they issue
  ~0.7us after the bass preamble instead of at tile-body entry. The PE
  is then continuously busy from ~6.7us, so the HAM activity monitor
  un-throttles the PE clock to 2.4GHz (~11.6us) BEFORE the first real
  matmul (~12.4us, gated by the xt DMA's completion receipt). A manual
  semaphore + a main-block PE wait_ge bridges the pre-context DMAs to
  the in-context matmuls (one wait covers all: PE executes in order).
- Input pieces stay >=300KB: each dma trigger serializes ~0.65us on the
  sync sequencer and the stream ramps 130->430GB/s, so finer pieces
  starve the ring and reset the HAM window (measured).
- Separate PSUM pools (L1=4, L2=2, RT=2 banks): L2-entry doesn't wait
  on L1 psum-buffer reuse.
- KPAIR (two half-array K=64 ko4 matmuls via tile_position) measured
  ZERO overlap on HW and bunches post-ops -> off.
- Chunk balance doesn't matter (measured): the MM stream costs
  ~sum(N_c)/2.4GHz + ~7ns/MM regardless of the split.
- Output DMAs ride the sync engine (idle after the input triggers,
  ~0.6us faster dispatch than gpsimd's Q7 path for the final flush),
  one merged DMA per outx chunk.
"""
import os
from contextlib import ExitStack

import numpy as np
import ml_dtypes

import concourse.bacc as bacc
import concourse.mybir as mybir
import concourse.tile as tile
from concourse.bass_utils import run_bass_kernel_spmd

F32 = mybir.dt.float32
F32R = mybir.dt.float32r
BF16 = mybir.dt.bfloat16
AFT = mybir.ActivationFunctionType

# problem dims (hardcoded per spec)
B, FEAT, EMB, HID, NCLS = 4096, 512, 64, 1024, 64
LIB = 256          # LIB_X == LIB_Y
OUTJ = 256
NCORES = 8
D_IN = FEAT + EMB          # 576
KO1 = 5                    # ceil(576/128) K-blocks for layer 1
D_PAD = KO1 * 128          # 640
KO2 = HID // 128           # 8
# Sized past the TYPICAL gating-DMA completion (~12.4us) with margin for
# slow-stream runs: if warmups end early and the stream is late, the PE
# idles, the HAM monitor re-throttles the clock to 1.2GHz mid-kernel and
# the run loses ~5-10us (measured). Extra warmups are nearly free on
# fast runs (HAM fires mid-warmup, the rest run at 2x clock).
N_WARMUP = 48              # dummy matmuls (N=128) to warm PE during loads

_DT_MAP = {"f32": F32, "f32r": F32R, "bf16": BF16}
_NP_MAP = {"f32": np.float32, "f32r": np.float32, "bf16": ml_dtypes.bfloat16}
DT_A_NAME = os.environ.get("KDT_A", "bf16")    # MLP weights/acts + NX path
DT_NY_NAME = os.environ.get("KDT_NY", "bf16")  # noise_lib_Y + cls_Y path
# ko4 (embed block) has only 64 live K rows: run two mo-chains' ko4 as
# concurrent K=64 matmuls in the two row-halves of the PE array
# measured: the two half-array ko4 matmuls run serially (no concurrency
# win) and pairing bunches post-ops, stalling L2 entry -> keep OFF
PAIR_KO4 = os.environ.get("KPAIR", "0") == "1"
N_WARMUP = int(os.environ.get("KWARM", str(N_WARMUP)))


def _round_up(a, b):
    return (a + b - 1) // b * b


def _plan(option):
    opt = np.asarray(option).astype(np.int64).ravel()
    assert opt.shape[0] == B
    g = np.bincount(opt, minlength=NCLS)
    order = np.argsort(opt, kind="stable")
    starts = np.concatenate([[0], np.cumsum(g)])
    # pseudo-classes: (class_id, sample idx array), each <= 128 samples
    pieces = []
    for m in range(NCLS):
        idx = order[starts[m]:starts[m + 1]]
        if len(idx) == 0:
            pieces.append((m, idx))
        for o in range(0, len(idx), 128):
            pieces.append((m, idx[o:o + 128]))
    pieces.sort(key=lambda t: -len(t[1]))
    while len(pieces) % NCORES:
        pieces.append((pieces[-1][0], np.empty(0, np.int64)))
    nslots = len(pieces) // NCORES

    caps = []                      # per-slot capacity (compile-time)
    sstart = []                    # per-slot column start
    cls_of = np.zeros((nslots, NCORES), np.int64)  # class in (slot, core)
    core_of = np.empty(B, np.int64)
    col_of = np.empty(B, np.int64)
    row_of = np.empty(B, np.int64)
    s = 0
    for j in range(nslots):
        grp = pieces[NCORES * j:NCORES * (j + 1)]
        cap = max(4, _round_up(len(grp[0][1]), 2))
        for c, (m, idx) in enumerate(grp):
            cls_of[j, c] = m
            if len(idx):
                core_of[idx] = c
                col_of[idx] = s + np.arange(len(idx))
                row_of[idx] = np.arange(len(idx))
        sstart.append(s)
        caps.append(cap)
        s += cap
    SU_pad = s
    rmax = max(caps)

    # column chunks (psum free dim <= 512), boundaries at slot starts.
    # (Measured: chunk balance doesn't matter - the MM stream cost is
    # ~sum(N_c)/2.4GHz either way, and a small tail chunk pipelines at
    # ~52ns/MM under FWL. Greedy max-width chunks measured best.)
    chunks = []
    c0 = 0
    for j in range(nslots):
        if sstart[j] + caps[j] - c0 > 512:
            chunks.append((c0, sstart[j]))
            c0 = sstart[j]
    chunks.append((c0, SU_pad))
    assert all(c1 - c0 <= 512 for c0, c1 in chunks)
    assert rmax <= 128

    return dict(opt=opt, caps=caps, sstart=sstart, nslots=nslots,
                SU_pad=SU_pad, rmax=rmax, chunks=chunks, cls_of=cls_of,
                core_of=core_of, col_of=col_of, row_of=row_of,
                has_bias=True)


_NC_CACHE = {}


def _build_nc(plan):
    DT_A = _DT_MAP[DT_A_NAME]
    DT_NY = _DT_MAP[DT_NY_NAME]
    SU_pad = plan["SU_pad"]
    chunks = plan["chunks"]
    caps = plan["caps"]
    sstart = plan["sstart"]
    nslots = plan["nslots"]
    rmax = plan["rmax"]
    has_bias = plan["has_bias"]

    key = (tuple(caps), tuple(chunks), has_bias, PAIR_KO4,
           DT_A_NAME, DT_NY_NAME)
    if key in _NC_CACHE:
        return _NC_CACHE[key]

    c_w1 = KO1 * HID
    c_w2 = KO2 * LIB
    c_bx = c_w1 + c_w2                 # X-branch blob columns
    NY_COLS = nslots * 2 * OUTJ

    ch0 = chunks[0][1]
    c_xt_a = KO1 * ch0
    c_xt_b = KO1 * SU_pad - c_xt_a
    c_mo = KO1 * 128           # one mo-block of W1

    nc = bacc.Bacc()
    # xt split: ko0-1 ship pre-context (small -> early completion sem,
    # gates the first chains); ko2-4 ship as an in-context tile whose
    # Tile-managed completion overlaps the first chains' execution. The
    # gate sem is limited by a straggler SDMA engine whose final inc
    # lags ~1.3us behind the other 15 (measured), so a smaller gating
    # piece starts the PE ~1us earlier.
    xta_d = nc.dram_tensor("xta", [128, 2 * SU_pad], DT_A,
                           kind="ExternalInput")
    xtb_d = nc.dram_tensor("xtb", [128, (KO1 - 2) * SU_pad], DT_A,
                           kind="ExternalInput")
    w1y_d = nc.dram_tensor("w1y", [128, c_w1], DT_A, kind="ExternalInput")
    # w1y piece boundaries (in mo blocks), 2 mo each. Pieces must stay
    # >=300KB: each dma trigger serializes ~0.65us on the sync sequencer
    # and the early stream runs only ~160-360GB/s, so finer pieces starve
    # the ring (measured: per-ko xt slices stalled L1 and kept resetting
    # the HAM warmup window -> cold until 20us).
    w1y_pieces = [(0, 2), (2, 4), (4, 6), (6, KO2)]
    w2y_d = nc.dram_tensor("w2y", [128, c_w2], DT_A, kind="ExternalInput")
    blobx_d = nc.dram_tensor("blobx", [128, c_bx], DT_A, kind="ExternalInput")
    blobx_pieces = [(0, 4 * c_mo), (4 * c_mo, c_bx)]  # w1x mo0-3 | rest
    bias_d = (nc.dram_tensor("bias", [128, 20], F32, kind="ExternalInput")
              if has_bias else None)
    ny_d = nc.dram_tensor("ny", [128, NY_COLS], DT_NY, kind="ExternalInput")
    outx_d = nc.dram_tensor("outx", [128, 2 * SU_pad], BF16,
                            kind="ExternalOutput")
    outy_d = nc.dram_tensor("outy", [rmax, nslots * OUTJ], BF16,
                            kind="ExternalOutput")

    # ---- pre-TileContext section: lands in the 'main' block, which the
    # engines execute right after the bass preamble (~5.6us), ~1.5us
    # before the tile body entry. The gating input DMAs trigger here, and
    # the PE warmup matmuls run here back-to-back so the HAM activity
    # window sees sustained PE busy from ~5.6us and un-throttles to
    # 2.4GHz before the first real matmul (~10.5us). The warmup operands
    # are uninitialized SBUF/PSUM garbage - never read downstream (every
    # consumer matmul group opens with start=True).
    early_sem = nc.alloc_semaphore("early_dma")
    w1y0_sb = nc.alloc_sbuf_tensor("w1y0_early", [128, 2 * c_mo], DT_A)
    xt_raw = nc.alloc_sbuf_tensor("xt_early", [128, 2 * SU_pad], DT_A)
    warm_in = nc.alloc_sbuf_tensor("warm_in", [128, 128], BF16)
    _psum_save = (nc.psum_base, nc.psum_top)
    warm_ps = nc.alloc_psum_tensor("warm_ps", [128, 128], F32)
    # let the tile psum pools overlap warm_ps: all warmup matmuls retire
    # before the first pool matmul (PE FIFO), which opens with start=True
    nc.psum_base, nc.psum_top = _psum_save
    nc.sync.dma_start(w1y0_sb.ap(), w1y_d[:, 0:2 * c_mo]).then_inc(early_sem, 16)
    nc.sync.dma_start(xt_raw.ap(), xta_d[:]).then_inc(early_sem, 16)
    for _ in range(N_WARMUP):
        nc.tensor.matmul(warm_ps.ap(), lhsT=warm_in.ap(), rhs=warm_in.ap(),
                         start=True, stop=True)
    # gate the PE stream on the gating DMAs here in the main block (the
    # tile scheduler's sim can't see pre-context sem increments and would
    # deadlock on an in-context wait). The PE executes its queue in
    # order, so every in-context matmul reading xt/w1y0 is covered.
    nc.tensor.wait_ge(early_sem, 32)
    w1y0_ap = w1y0_sb.ap()
    xt_ap = xt_raw.ap()

    with tile.TileContext(nc) as tc, ExitStack() as ctx:
        const = ctx.enter_context(tc.tile_pool(name="const", bufs=1))
        act = ctx.enter_context(tc.tile_pool(name="act", bufs=1))
        hpool = ctx.enter_context(tc.tile_pool(name="hpool", bufs=2))
        l1_ps = ctx.enter_context(tc.tile_pool(name="l1_ps", bufs=4, space="PSUM"))
        l2_ps = ctx.enter_context(tc.tile_pool(name="l2_ps", bufs=2, space="PSUM"))
        rt_ps = ctx.enter_context(tc.tile_pool(name="rt_ps", bufs=2, space="PSUM"))

        # remaining input DMAs, all on the sync-engine HWDGE ring (one HW
        # queue, data flows in trigger order at ~300GB/s). w1y piece0 and
        # xt were already triggered pre-context; the ring keeps order, so
        # these stream right behind them in consumption order. The sync
        # engine reaches these triggers only at tile-body entry (gated by
        # the PE's pre-context warmup, ~10.5us) - still well ahead of each
        # consumer (w2y needed ~17.5us, ny ~20us, blobx ~22/26us).
        # xt ko2-4: FIRST in-context trigger (ring position right behind
        # the pre-context pieces; chain mo0 hits ko2 ~0.5us after start)
        xtb_sb = const.tile([128, (KO1 - 2) * SU_pad], DT_A, tag="xtb",
                            name="xtb")
        nc.sync.dma_start(xtb_sb[:], xtb_d[:])
        xtb_v = xtb_sb.rearrange("p (ko b) -> p ko b", ko=KO1 - 2)

        w1_tiles = {"y": [None] * KO2, "x": None}
        w1y_sbs = {}
        for lo, hi in w1y_pieces[1:]:
            t = const.tile([128, (hi - lo) * c_mo], DT_A, tag=f"w1y{lo}",
                           name=f"w1y{lo}")
            w1y_sbs[lo] = t
            v = t.rearrange("p (mo ko m) -> p mo ko m", mo=hi - lo, ko=KO1)
            for mo in range(lo, hi):
                w1_tiles["y"][mo] = v[:, mo - lo]
            nc.sync.dma_start(t[:], w1y_d[:, lo * c_mo:hi * c_mo])
        w2y_sb = const.tile([128, c_w2], DT_A)
        nc.sync.dma_start(w2y_sb[:], w2y_d[:])
        if has_bias:
            bias_sb = const.tile([128, 20], F32)
            nc.sync.dma_start(bias_sb[:], bias_d[:])
        ny_sb = const.tile([128, NY_COLS], DT_NY)
        nc.sync.dma_start(ny_sb[:], ny_d[:])
        ny_v = ny_sb.rearrange("p (s ko j) -> p s ko j", s=nslots, ko=2)
        blobx_sb = const.tile([128, c_bx], DT_A)
        for lo, hi in blobx_pieces:
            nc.sync.dma_start(blobx_sb[:, lo:hi], blobx_d[:, lo:hi])
        w1_tiles["x"] = blobx_sb[:, 0:c_w1].rearrange(
            "p (mo ko m) -> p mo ko m", mo=KO2, ko=KO1)
        w2x_sb = blobx_sb[:, c_w1:c_w1 + c_w2]

        def xtv(ko, c0, c1):
            if ko < 2:
                return xt_ap[:, ko * SU_pad + c0:ko * SU_pad + c1]
            return xtb_v[:, ko - 2, c0:c1]

        def w1v(br, mo, ko, msl):
            if br == "y":
                if mo < 2:
                    base = mo * KO1 * 128 + ko * 128
                    return w1y0_ap[:, base + msl.start:base + msl.stop]
                return w1_tiles["y"][mo][:, ko, msl]
            return w1_tiles["x"][:, mo, ko, msl]

        w2_v = {"y": w2y_sb.rearrange("p (ko m) -> p ko m", ko=KO2),
                "x": w2x_sb.rearrange("p (ko m) -> p ko m", ko=KO2)}
        # bias cols: b1y[0:8] b2y[8:10] b1x[10:18] b2x[18:20]
        bcol = {"y": (0, 8), "x": (10, 18)}

        outy_sb = act.tile([128, nslots, OUTJ], BF16, tag="outy")
        outy_dv = outy_d.rearrange("p (s j) -> p s j", s=nslots)
        outxT = act.tile([128, 2, SU_pad], BF16, tag="outxT")
        outx_dv = outx_d.rearrange("p (jo b) -> p jo b", jo=2)
        clsy = act.tile([128, 2, SU_pad], DT_NY, tag="clsy", name="clsy")
        CH_MAX = max(c1 - c0 for c0, c1 in chunks)

        def post_op(idx, out, ps, func, bias_ap):
            # alternate ScalarE/VectorE per index: splits the psum-drain
            # load across both engines so neither gates the PE.
            if idx % 2 == 0:
                if not has_bias and func is AFT.Identity:
                    nc.scalar.copy(out, ps)
                else:
                    nc.scalar.activation(out, ps, func,
                                         bias=bias_ap if has_bias else 0.0)
            elif func is AFT.Relu:
                if has_bias:
                    nc.vector.tensor_scalar(out, ps, bias_ap, 0.0,
                                            mybir.AluOpType.add,
                                            mybir.AluOpType.max)
                else:
                    nc.vector.tensor_scalar(out, ps, 0.0, None,
                                            mybir.AluOpType.max)
            elif has_bias:
                nc.vector.tensor_scalar(out, ps, bias_ap, None,
                                        mybir.AluOpType.add)
            else:
                nc.vector.tensor_copy(out, ps)

        def mlp(br, after_chunk=None):
            # chunk-major: both layers of chunk ci complete before ci+1,
            # so per-chunk consumers (routing / outx DMA) start early.
            h_sb = hpool.tile([128, KO2, SU_pad], DT_A, tag="h", name=f"h_{br}")
            b1o, b2o = bcol[br]
            for ci, (c0, c1) in enumerate(chunks):
                cw = c1 - c0
                if PAIR_KO4:
                    for t in range(KO2 // 2):
                        moa, mob = 2 * t, 2 * t + 1
                        pss = []
                        for mo in (moa, mob):
                            ps = l1_ps.tile([128, CH_MAX], F32, tag="l1",
                                            name="l1_ps_t")[:, :cw]
                            pss.append(ps)
                            for ko in range(KO1 - 1):
                                nc.tensor.matmul(
                                    ps, lhsT=w1v(br, mo, ko, slice(0, 128)),
                                    rhs=xtv(ko, c0, c1),
                                    start=(ko == 0), stop=False)
                        # ko4: both mo chains' embed blocks concurrently in
                        # the two row-halves of the PE (weights for mob are
                        # packed into rows 64:128 of moa's ko4 block)
                        wa = w1v(br, moa, KO1 - 1, slice(0, 128))
                        xr = xtv(KO1 - 1, c0, c1)
                        for hi, ps in enumerate(pss):
                            h0 = 64 * hi
                            nc.tensor.matmul(
                                ps, lhsT=wa[h0:h0 + 64, :],
                                rhs=xr[h0:h0 + 64, :],
                                start=False, stop=True,
                                tile_position=(h0, 0))
                        for hi, mo in enumerate((moa, mob)):
                            post_op(mo, h_sb[:, mo, c0:c1], pss[hi], AFT.Relu,
                                    bias_sb[:, b1o + mo:b1o + mo + 1]
                                    if has_bias else None)
                else:
                    mo_start = 0
                    if br == "y" and ci == 0:
                        # interleave mo0/mo1's ko0-1 (they read only the
                        # pre-context xta/w1y0 pieces) so the xtb tile's
                        # DMA-completion straggler overlaps real matmuls
                        # instead of stalling the mo0 chain at ko2
                        mo_start = 2
                        pss = [l1_ps.tile([128, CH_MAX], F32, tag="l1",
                                          name="l1_ps_t")[:, :cw]
                               for _ in range(2)]
                        for ko in range(2):
                            for mi in range(2):
                                nc.tensor.matmul(
                                    pss[mi],
                                    lhsT=w1v(br, mi, ko, slice(0, 128)),
                                    rhs=xtv(ko, c0, c1),
                                    start=(ko == 0), stop=False)
                        for mi in range(2):
                            for ko in range(2, KO1):
                                nc.tensor.matmul(
                                    pss[mi],
                                    lhsT=w1v(br, mi, ko, slice(0, 128)),
                                    rhs=xtv(ko, c0, c1),
                                    start=False, stop=(ko == KO1 - 1))
                            post_op(mi, h_sb[:, mi, c0:c1], pss[mi],
                                    AFT.Relu,
                                    bias_sb[:, b1o + mi:b1o + mi + 1]
                                    if has_bias else None)
                    for mo in range(mo_start, KO2):
                        ps = l1_ps.tile([128, CH_MAX], F32, tag="l1",
                                        name="l1_ps_t")[:, :cw]
                        for ko in range(KO1):
                            nc.tensor.matmul(
                                ps, lhsT=w1v(br, mo, ko, slice(0, 128)),
                                rhs=xtv(ko, c0, c1),
                                start=(ko == 0), stop=(ko == KO1 - 1))
                        post_op(mo, h_sb[:, mo, c0:c1], ps, AFT.Relu,
                                bias_sb[:, b1o + mo:b1o + mo + 1]
                                if has_bias else None)
                for jo in range(2):
                    ps = l2_ps.tile([128, CH_MAX], F32, tag="l2",
                                    name="l2_ps_t")[:, :cw]
                    for ko in range(KO2):
                        nc.tensor.matmul(
                            ps, lhsT=w2_v[br][:, ko, jo * 128:(jo + 1) * 128],
                            rhs=h_sb[:, ko, c0:c1],
                            start=(ko == 0), stop=(ko == KO2 - 1))
                    tgt = clsy if br == "y" else outxT
                    post_op(jo, tgt[:, jo, c0:c1], ps, AFT.Identity,
                            bias_sb[:, b2o + jo:b2o + jo + 1]
                            if has_bias else None)
                if br == "x":
                    # one DMA per chunk covering both jo halves, issued from
                    # the sync engine (idle once the input triggers retire
                    # ~14us; HWDGE dispatch is ~0.6us faster than gpsimd's
                    # Q7 path, which matters for the kernel-final flush)
                    nc.sync.dma_start(outx_dv[:, :, c0:c1],
                                      outxT[:, :, c0:c1])
                if br != "x" and after_chunk is not None:
                    after_chunk(ci, c1)

        # routing: slot j holds one whole class (per core); out_Y rows =
        # clsY[:, s:s+c].T @ NY[slot j]  -- one full-width matmul pair.
        # Emitted per chunk as soon as that chunk's clsY is ready; copies
        # alternate ScalarE/VectorE so psum drains don't gate the PE.
        rt_done = [0]

        def route_upto(ci, c1):
            last = ci == len(chunks) - 1
            j0 = rt_done[0]
            j = j0
            while j < nslots and (last or sstart[j] + caps[j] <= c1):
                ps = rt_ps.tile([128, OUTJ], F32, tag="rt", name="rt_ps_t")
                for ko in range(2):
                    nc.tensor.matmul(
                        ps[0:caps[j], :],
                        lhsT=clsy[:, ko, sstart[j]:sstart[j] + caps[j]],
                        rhs=ny_v[:, j, ko, :],
                        start=(ko == 0), stop=(ko == 1))
                if j % 2 == 0:
                    nc.scalar.copy(outy_sb[:, j, :], ps[:])
                else:
                    nc.vector.tensor_copy(outy_sb[:, j, :], ps[:])
                j += 1
            rt_done[0] = j
            if j > j0:
                nc.sync.dma_start(outy_dv[:, j0:j, :],
                                  outy_sb[0:rmax, j0:j, :])

        mlp("y", after_chunk=route_upto)
        mlp("x")   # out_X comes straight from the fused MLP2-X (DMA inside)

    nc.compile()
    _NC_CACHE[key] = nc
    return nc


def _prepare_inputs(plan, state, option, embed_table, Wx1, bx1, Wx2, bx2,
                    Wy1, by1, Wy2, by2, noise_lib_X, noise_lib_Y):
    np_a = _NP_MAP[DT_A_NAME]
    np_ny = _NP_MAP[DT_NY_NAME]
    SU_pad = plan["SU_pad"]
    opt = plan["opt"]
    nslots = plan["nslots"]
    core_of, col_of = plan["core_of"], plan["col_of"]
    cls_of = plan["cls_of"]

    state = np.asarray(state, np.float32)
    embed_table = np.asarray(embed_table, np.float32)

    # per-core feature-major inputs
    Xall = np.zeros((NCORES, SU_pad, D_PAD), np.float32)
    Xall[core_of, col_of, :FEAT] = state
    Xall[core_of, col_of, FEAT:D_IN] = embed_table[opt]
    if PAIR_KO4:
        # duplicate embed rows into the ko4 zero-pad so the two row-half
        # K=64 matmuls (mo pair) both see the embed features
        Xall[core_of, col_of, D_IN:D_IN + EMB] = embed_table[opt]
    # [NCORES, 128, KO1, SU_pad]
    xt = Xall.transpose(0, 2, 1).reshape(NCORES, KO1, 128, SU_pad) \
        .transpose(0, 2, 1, 3).astype(np_a)
    xt = np.ascontiguousarray(xt).reshape(NCORES, 128, -1)

    def pack_w1(w):
        # mo-major: [128p, mo, ko, 128] flattened
        w = np.asarray(w, np.float32)
        wp = np.zeros((D_PAD, HID), np.float32)
        wp[:D_IN] = w
        if PAIR_KO4:
            # even mo's ko4 block rows 64:128 carry mo+1's embed weights
            # (consumed by the row-half-64 matmul of the pair)
            for mo in range(0, KO2, 2):
                wp[D_IN:D_IN + EMB, mo * 128:(mo + 1) * 128] = \
                    w[FEAT:D_IN, (mo + 1) * 128:(mo + 2) * 128]
        return wp.reshape(KO1, 128, KO2, 128).transpose(1, 2, 0, 3) \
            .reshape(128, KO1 * HID)

    def pack_w2(w):
        return np.asarray(w, np.float32).reshape(KO2, 128, LIB) \
            .transpose(1, 0, 2).reshape(128, KO2 * LIB)

    nxf = np.asarray(noise_lib_X, np.float64)
    w2x_fused = (np.asarray(Wx2, np.float64) @ nxf).astype(np.float32)
    b2x_fused = (np.asarray(bx2, np.float64) @ nxf).astype(np.float32)
    w1y = np.ascontiguousarray(pack_w1(Wy1).astype(np_a))
    w2y = np.ascontiguousarray(pack_w2(Wy2).astype(np_a))
    blobx = np.ascontiguousarray(np.concatenate(
        [pack_w1(Wx1), pack_w2(w2x_fused)], axis=1).astype(np_a))

    bias = np.zeros((128, 20), np.float32)
    bias[:, 0:8] = np.asarray(by1, np.float32).reshape(8, 128).T
    bias[:, 8:10] = np.asarray(by2, np.float32).reshape(2, 128).T
    bias[:, 10:18] = np.asarray(bx1, np.float32).reshape(8, 128).T
    bias[:, 18:20] = b2x_fused.reshape(2, 128).T

    # ny per core: [128, slot, ko, OUTJ] - slot j carries class cls_of[j, c]
    nyf = np.asarray(noise_lib_Y, np.float32)  # [NCLS, 256, 256]
    ny = np.empty((NCORES, 128, nslots, 2, OUTJ), np.float32)
    for c in range(NCORES):
        sel = nyf[cls_of[:, c]]                       # [nslots, 256, 256]
        ny[c] = sel.reshape(nslots, 2, 128, OUTJ).transpose(2, 0, 1, 3)
    ny = np.ascontiguousarray(ny.reshape(NCORES, 128, -1).astype(np_ny))

    in_maps = []
    for c in range(NCORES):
        m = {"xta": np.ascontiguousarray(xt[c][:, :2 * SU_pad]),
             "xtb": np.ascontiguousarray(xt[c][:, 2 * SU_pad:]),
             "w1y": w1y, "w2y": w2y,
             "blobx": blobx, "ny": ny[c]}
        if plan["has_bias"]:
            m["bias"] = bias
        in_maps.append(m)
    return in_maps


def _gather_outputs(plan, results):
    core_of, col_of, row_of = (plan["core_of"], plan["col_of"],
                               plan["row_of"])
    nslots = plan["nslots"]
    # slot of each sample from its column
    sstart = np.asarray(plan["sstart"] + [plan["SU_pad"]])
    slot_of = np.searchsorted(sstart, col_of, side="right") - 1
    ox = np.stack([np.asarray(r["outx"]) for r in results])  # [8,128,2*SU]
    oy = np.stack([np.asarray(r["outy"]) for r in results])  # [8,rmax,ns*J]
    ox = ox.reshape(NCORES, 128, 2, plan["SU_pad"])
    oy = oy.reshape(NCORES, plan["rmax"], nslots, OUTJ)
    gx = np.empty((B, 2 * 128), np.float32)
    gx[:, :128] = ox[core_of, :, 0, col_of]
    gx[:, 128:] = ox[core_of, :, 1, col_of]
    gy = oy[core_of, row_of, slot_of].astype(np.float32)
    return gx, gy


def _run(inputs, trace=False):
    plan = _plan(inputs["option"])
    plan["has_bias"] = any(
        np.any(np.asarray(inputs[k])) for k in ("bx1", "bx2", "by1", "by2"))
    nc = _build_nc(plan)
    in_maps = _prepare_inputs(plan, **inputs)
    res = run_bass_kernel_spmd(nc, in_maps, core_ids=list(range(NCORES)),
                               trace=trace)
    gx, gy = _gather_outputs(plan, res.results)
    return (gx, gy), res


def kernel(**inputs):
    (gx, gy), _ = _run(inputs, trace=False)
    return gx, gy

